# revision 27
# baseline (speedup 1.0000x reference)
"""ARMIN/TARDIS addressed-memory cell on 8 trn2 NeuronCores.

Score phase: data-parallel over batch (32 rows/core). The big content-
addressing matmul runs as a SINGLE float32r pass (1 cyc/row, ~1e-4
score error) instead of the 3-pass bf16 hi/lo split; the argmax is then
made exact by re-scoring the top-4 candidate slots per row with fp32
matmuls on gathered hmem rows (an augmented DRAM copy carries the key
row and -100*prev alongside each hmem row so one indirect gather feeds
the whole exact re-score). The LN-sigma of the mixed score vector is
accurate to ~1e-5 rel, 15x+ below the minimum argmax margin.

Cell phase: 8-way column split. Each core computes the full 256-row
batch for its own 128 columns of every LN chunk of W_full1/W_full
(4.6MB of weights per core instead of 36MB, prefetched during the
score phase). h_entry and the transposed gate are AllGathered; LN
statistics are AllReduced as per-chunk partial sums.
"""

import numpy as np
import ml_dtypes
from contextlib import ExitStack

import concourse.bass as bass
import concourse.bacc as bacc
import concourse.tile as tile
from concourse import mybir
from concourse.bass_utils import run_bass_kernel_spmd
from concourse.masks import make_identity

F32 = mybir.dt.float32
F32R = mybir.dt.float32r
BF16 = mybir.dt.bfloat16
U32 = mybir.dt.uint32
AF = mybir.ActivationFunctionType
ALU = mybir.AluOpType
AX = mybir.AxisListType

N_CORES = 8
B, X, H, M, KD = 256, 512, 1024, 128, 64
BC = B // N_CORES        # 32 batch rows per core
F = H // 4               # 256
BM = BC * M              # 4096
EPS = 1e-5
F_BIAS = 1.0
CHUNK = 512              # bm columns per score-path tile
NCHUNK = BM // CHUNK     # 8
NB = CHUNK // M          # batch rows per chunk (4)
KCAT = (X + 2 * H) // 128  # 20 contraction tiles for the cell matmuls
NAUG = 10                # hmem_aug width in 128-col tiles (hmem 8, keys 1, prev 1)
KSC = 4                  # candidates refined per row
GRP = [list(range(N_CORES))]


def _bcast_rows(handle_ap, lo, hi, rows=BC):
    """AP that reads dram vector[lo:hi] broadcast across `rows` partitions."""
    src = handle_ap[lo:hi]
    return bass.AP(tensor=src.tensor, offset=src.offset,
                   ap=[[0, rows]] + [list(d) for d in src.ap])


def build_nc():
    nc = bacc.Bacc("TRN2", target_bir_lowering=False, debug=False,
                   num_devices=N_CORES)
    P = {}

    def dp(name, shape, dtype=F32, out=False):
        P[name] = nc.declare_dram_parameter(name, list(shape), dtype, isOutput=out)
        return P[name]

    dp("hmemT", [H, BM], F32R)       # [h, b*128+m] fp32 bits, f32r dtype
    dp("hmem_aug", [BM, NAUG * 128])  # rows: hmem | keys_pad | (-100*prev, 0...)
    dp("xT", [X, BC]); dp("cT", [H, BC])
    dp("u_t", [BC, M]); dp("prev", [BC, M]); dp("gumbel_u", [BC, M])
    dp("keysT_pad", [128, M])
    dp("fcxc", [X + H, F])           # fc_w rows 0:1536
    dp("fchx", [H + 128, F])         # fc_w rows 1600:2624 then padded 1536:1600
    dp("fchx_r", [H, F], F32R)       # same bits as fchx[0:H], f32r dtype
    dp("fcu", [128, F])              # fc_w rows 2624:2752
    dp("fcbp", [F]); dp("veca", [F]); dp("veca_r", [F], F32R)
    dp("iota_m", [M])
    dp("row_base", [BC, 1], U32)
    dp("w1_cols", [X + 2 * H, 2 * 128], BF16)
    dp("wf_cols", [X + 2 * H, 5 * 128], BF16)
    dp("xT_bf_full", [X, B], BF16)
    dp("cT_bf_full", [H, B], BF16)
    dp("c_cols", [B, 128])
    dp("s_sel", [H, 128])
    out_d = dp("out", [B, 256], out=True)
    dbg_d = dp("dbg_s", [BC, M], out=True)   # f32r base scores (pre-LN)

    score_d = nc.dram_tensor("score_bounce", [BM], F32)
    off_bounce = nc.dram_tensor("off_bounce", [KSC * BC], U32)
    sc_bounce = nc.dram_tensor("sc_bounce", [KSC * BC], F32)
    agi_in = nc.dram_tensor("agi_in", [BC, 1], U32)
    agi_out = nc.dram_tensor("agi_out", [B, 1], U32, addr_space="Shared")
    ag_g1_in = nc.dram_tensor("ag_g1_in", [256, B], BF16)
    ag_g1_out = nc.dram_tensor("ag_g1_out", [2048, B], BF16, addr_space="Shared")
    ar3_in = nc.dram_tensor("ar3_in", [B, 4], F32)
    ar3_out = nc.dram_tensor("ar3_out", [B, 4], F32, addr_space="Shared")
    ar1_in = nc.dram_tensor("ar1_in", [B, 10], F32)
    ar1_out = nc.dram_tensor("ar1_out", [B, 10], F32, addr_space="Shared")
    ar2_in = nc.dram_tensor("ar2_in", [B, 2], F32)
    ar2_out = nc.dram_tensor("ar2_out", [B, 2], F32, addr_space="Shared")
    arw_in = nc.dram_tensor("arw_in", [1, 8], F32)
    arw_out = nc.dram_tensor("arw_out", [1, 8], F32, addr_space="Shared")

    with ExitStack() as ctx:
        tc = ctx.enter_context(tile.TileContext(nc))
        consts = ctx.enter_context(tc.tile_pool(name="consts", bufs=1))
        hpool = ctx.enter_context(tc.tile_pool(name="hpool", bufs=3))
        qrpool = ctx.enter_context(tc.tile_pool(name="qrpool", bufs=2))
        hfpool = ctx.enter_context(tc.tile_pool(name="hfpool", bufs=2))
        bnpool = ctx.enter_context(tc.tile_pool(name="bnpool", bufs=4))
        scpool = ctx.enter_context(tc.tile_pool(name="scpool", bufs=2))
        pre_ps = ctx.enter_context(tc.tile_pool(name="pre_ps", bufs=2, space="PSUM"))
        tp_ps = ctx.enter_context(tc.tile_pool(name="tp_ps", bufs=2, space="PSUM"))
        ps1_ps = ctx.enter_context(tc.tile_pool(name="ps1_ps", bufs=2, space="PSUM"))
        ps2_ps = ctx.enter_context(tc.tile_pool(name="ps2_ps", bufs=1, space="PSUM"))

        # ---------- resident constants ----------
        ident = consts.tile([128, 128], F32, tag="ident")
        make_identity(nc, ident[:])
        eps_t = consts.tile([BC, 1], F32, tag="eps")
        nc.vector.memset(eps_t[:], EPS)
        eps128 = consts.tile([128, 1], F32, tag="eps128")
        nc.vector.memset(eps128[:], EPS)
        e20_t = consts.tile([BC, 1], F32, tag="e20")
        nc.vector.memset(e20_t[:], 1e-20)
        magic = consts.tile([128, 1], U32, tag="magic")
        nc.vector.memset(magic[:], 0x5f3759df)
        warm = consts.tile([1, 8], F32, tag="warm")
        nc.vector.memset(warm[:], 1.0)
        nc.gpsimd.dma_start(out=arw_in.ap()[:], in_=warm[:])
        nc.gpsimd.collective_compute(
            "AllReduce", ALU.add, replica_groups=GRP,
            ins=[arw_in.ap()[:]], outs=[arw_out.ap()[:]])

        def rsqrt_newton(dst_ap, x_ap, tmp_shape, tag):
            """dst = 1/sqrt(x) on the vector engine (no ACT table).
            x must be > 0. ~5e-6 rel err after one step, ~1e-11 after two."""
            p = tmp_shape[0]
            t1 = bnpool.tile(tmp_shape, U32, tag=f"{tag}_u", name="rsq_u")
            nc.vector.tensor_scalar(out=t1[:], in0=x_ap.bitcast(U32), scalar1=1,
                                    scalar2=None, op0=ALU.logical_shift_right)
            y = dst_ap
            if len(tmp_shape) == 2:
                mg = magic[:p, 0:1].to_broadcast(tmp_shape)
            else:
                mg = magic[:p, None, 0:1].to_broadcast(tmp_shape)
            nc.vector.tensor_tensor(out=y.bitcast(U32), in0=mg, in1=t1[:],
                                    op=ALU.subtract)
            h = bnpool.tile(tmp_shape, F32, tag=f"{tag}_h", name="rsq_h")
            for _ in range(2):
                nc.vector.tensor_tensor(out=h[:], in0=y, in1=y, op=ALU.mult)
                nc.vector.tensor_tensor(out=h[:], in0=h[:], in1=x_ap, op=ALU.mult)
                nc.vector.tensor_scalar(out=h[:], in0=h[:], scalar1=-0.5,
                                        scalar2=1.5, op0=ALU.mult, op1=ALU.add)
                nc.vector.tensor_tensor(out=y, in0=y, in1=h[:], op=ALU.mult)

        fchx = consts.tile([128, 9, F], F32, tag="fchx")
        nc.sync.dma_start(out=fchx[:, 8, :], in_=P["fchx"].ap()[H:H + 128, :])
        fchx_r = consts.tile([128, 8, F], F32R, tag="fchx_r")
        nc.sync.dma_start(out=fchx_r[:], in_=P["fchx_r"].ap()[:, :]
                          .rearrange("(j p) n -> p j n", p=128))
        veca_r = consts.tile([128, 2], F32R, tag="veca_r")
        nc.sync.dma_start(out=veca_r[:], in_=P["veca_r"].ap()
                          .rearrange("(f p) -> p f", p=128))
        fcxc = consts.tile([128, 12, F], F32, tag="fcxc")
        nc.sync.dma_start(out=fcxc[:], in_=P["fcxc"].ap()[:, :]
                          .rearrange("(j p) n -> p j n", p=128))
        fcu = consts.tile([128, F], F32, tag="fcu")
        nc.sync.dma_start(out=fcu[:], in_=P["fcu"].ap()[:])
        fcb = consts.tile([128, 2], F32, tag="fcb")
        nc.sync.dma_start(out=fcb[:], in_=P["fcbp"].ap().rearrange("(f p) -> p f", p=128))
        veca = consts.tile([128, 2], F32, tag="veca")
        nc.sync.dma_start(out=veca[:], in_=P["veca"].ap().rearrange("(f p) -> p f", p=128))
        keysT = consts.tile([128, M], F32, tag="keysT")
        nc.sync.dma_start(out=keysT[:], in_=P["keysT_pad"].ap()[:])
        xT_f = consts.tile([128, 4, BC], F32, tag="xT_f")
        nc.sync.dma_start(out=xT_f[:], in_=P["xT"].ap()[:, :]
                          .rearrange("(j p) n -> p j n", p=128))
        cT_f = consts.tile([128, 8, BC], F32, tag="cT_f")
        nc.sync.dma_start(out=cT_f[:], in_=P["cT"].ap()[:, :]
                          .rearrange("(j p) n -> p j n", p=128))
        u_sb = consts.tile([BC, M], F32, tag="u_sb")
        nc.sync.dma_start(out=u_sb[:], in_=P["u_t"].ap()[:])
        prev_sb = consts.tile([BC, M], F32, tag="prev_sb")
        nc.gpsimd.dma_start(out=prev_sb[:], in_=P["prev"].ap()[:])
        gum_sb = consts.tile([BC, M], F32, tag="gum_sb")
        nc.gpsimd.dma_start(out=gum_sb[:], in_=P["gumbel_u"].ap()[:])
        rowb = consts.tile([BC, 1], U32, tag="rowb")
        nc.gpsimd.dma_start(out=rowb[:], in_=P["row_base"].ap()[:])
        iota = consts.tile([BC, M], F32, tag="iota")
        nc.gpsimd.dma_start(out=iota[:], in_=_bcast_rows(P["iota_m"].ap(), 0, M))

        # ---------- u_norm and its transpose ----------
        usq = consts.tile([BC, M], F32, tag="usq")
        nc.scalar.activation(out=usq[:], in_=u_sb[:], func=AF.Square)
        nsq = consts.tile([BC, 1], F32, tag="nsq")
        nc.vector.reduce_sum(out=nsq[:], in_=usq[:], axis=AX.X)
        nc.vector.tensor_scalar_max(nsq[:], nsq[:], 1e-24)
        nrm = consts.tile([BC, 1], F32, tag="nrm")
        rsqrt_newton(nrm[:], nsq[:], [BC, 1], "nrm")
        unorm = consts.tile([BC, M], F32, tag="unorm")
        nc.vector.tensor_scalar_mul(unorm[:], u_sb[:], nrm[:])
        tp = tp_ps.tile([128, BC], F32, tag="tp")
        nc.tensor.transpose(tp[:], unorm[:], ident[:BC, :BC])
        unT = consts.tile([128, BC], F32, tag="unT")
        nc.vector.tensor_copy(out=unT[:], in_=tp[:])

        # ---------- q = xc @ W_xc + u_norm @ W_u  (natural [b, f], fp32) ----------
        qps = tp_ps.tile([BC, F], F32, tag="tp", name="qps")
        for k in range(4):
            nc.tensor.matmul(qps[:], lhsT=xT_f[:, k, :], rhs=fcxc[:, k, :],
                             start=(k == 0), stop=False)
        for k in range(8):
            nc.tensor.matmul(qps[:], lhsT=cT_f[:, k, :], rhs=fcxc[:, 4 + k, :],
                             start=False, stop=False)
        nc.tensor.matmul(qps[:], lhsT=unT[:], rhs=fcu[:], start=False, stop=True)
        q_nat = consts.tile([BC, F], F32, tag="q_nat")
        nc.vector.tensor_copy(out=q_nat[:], in_=qps[:])
        qT = consts.tile([128, 2, BC], F32, tag="qT")
        for f in range(2):
            tpq = tp_ps.tile([128, BC], F32, tag="tp", name="tpq")
            nc.tensor.transpose(tpq[:], q_nat[:, f * 128:(f + 1) * 128],
                                ident[:BC, :BC])
            nc.vector.tensor_copy(out=qT[:, f, :], in_=tpq[:])

        # ---------- r_km^T [f, m] = W_k_pad.T @ keysT_pad  (fp32) ----------
        rkT = consts.tile([128, 2, M], F32, tag="rkT")
        for f in range(2):
            rps = tp_ps.tile([128, M], F32, tag="tp", name="rps")
            nc.tensor.matmul(rps[:], lhsT=fchx[:, 8, f * 128:(f + 1) * 128],
                             rhs=keysT[:], start=True, stop=True)
            nc.vector.tensor_copy(out=rkT[:, f, :], in_=rps[:])

        # gumbel term and prev*100, precomputed off the critical tail
        gt = consts.tile([BC, M], F32, tag="gt")
        nc.scalar.activation(out=gt[:], in_=gum_sb[:], func=AF.Ln, bias=e20_t[:])
        nc.vector.tensor_scalar(out=gt[:], in0=gt[:], scalar1=-1.0, scalar2=1e-20,
                                op0=ALU.mult, op1=ALU.add)
        nc.scalar.activation(out=gt[:], in_=gt[:], func=AF.Ln)
        p100 = consts.tile([BC, M], F32, tag="p100")
        nc.vector.tensor_scalar_mul(p100[:], prev_sb[:], 100.0)

        s_bm = consts.tile([BC, M], F32, tag="s_bm")

        # ---------- score phase: one f32r pass per chunk ----------
        def issue_bhalf(ci):
            cs2 = slice(ci * CHUNK, (ci + 1) * CHUNK)
            htt = hpool.tile([128, 4, CHUNK], F32R, tag="ht", name="htb")
            nc.scalar.dma_start(out=htt[:],
                                in_=P["hmemT"].ap()[512:1024, cs2]
                                .rearrange("(j p) n -> p j n", p=128))
            return htt

        pend_b = issue_bhalf(0)
        for ci in range(NCHUNK):
            cs = slice(ci * CHUNK, (ci + 1) * CHUNK)
            hta = hpool.tile([128, 4, CHUNK], F32R, tag="ht", name="hta")
            nc.sync.dma_start(out=hta[:],
                              in_=P["hmemT"].ap()[0:512, cs]
                              .rearrange("(j p) n -> p j n", p=128))
            hthalf = [hta, pend_b]
            sps = tp_ps.tile([1, CHUNK], F32, tag="tp", name="sps")
            for f in range(2):
                fs = slice(f * 128, (f + 1) * 128)
                qr = qrpool.tile([128, CHUNK], F32, tag="qr", name="qr")
                nc.vector.tensor_tensor(
                    out=qr[:].rearrange("p (b m) -> p b m", b=NB),
                    in0=qT[:, f, ci * NB:(ci + 1) * NB, None].to_broadcast(
                        [128, NB, M]),
                    in1=rkT[:, f, None, :].to_broadcast([128, NB, M]),
                    op=ALU.add)
                ps = pre_ps.tile([128, CHUNK], F32, tag="pre", name="pre")
                for kh in range(8):
                    nc.tensor.matmul(ps[:],
                                     lhsT=fchx_r[:, kh, fs],
                                     rhs=hthalf[kh // 4][:, kh % 4, :],
                                     start=(kh == 0), stop=(kh == 7))
                hf = hfpool.tile([128, CHUNK], F32R, tag="hf", name="hf")
                nc.vector.tensor_tensor(out=hf[:], in0=ps[:], in1=qr[:], op=ALU.add)
                nc.scalar.activation(out=hf[:], in_=hf[:], func=AF.Tanh,
                                     bias=fcb[:, f:f + 1], scale=1.0)
                nc.tensor.matmul(sps[:], lhsT=veca_r[:, f:f + 1],
                                 rhs=hf[:],
                                 start=(f == 0), stop=(f == 1))
            if ci + 1 < NCHUNK:
                pend_b = issue_bhalf(ci + 1)
            scs = hfpool.tile([1, CHUNK], F32, tag="scs", name="scs")
            nc.vector.tensor_copy(out=scs[:], in_=sps[:])
            nc.gpsimd.dma_start(
                out=score_d.ap()[ci * CHUNK:(ci + 1) * CHUNK]
                .rearrange("(a n) -> a n", a=1),
                in_=scs[:])

        # exact-pass hmem weights: streamed after the hmem halves
        nc.sync.dma_start(out=fchx[:, 0:8, :], in_=P["fchx"].ap()[0:H, :]
                          .rearrange("(j p) n -> p j n", p=128))

        # ---------- cell-phase constant loads (stream during score) ----------
        w1sb = consts.tile([128, KCAT, 256], BF16, tag="w1sb")
        nc.sync.dma_start(out=w1sb[:], in_=P["w1_cols"].ap()[:, :]
                          .rearrange("(j p) n -> p j n", p=128))
        wfsb = consts.tile([128, KCAT, 640], BF16, tag="wfsb")
        nc.sync.dma_start(out=wfsb[:], in_=P["wf_cols"].ap()[:, :]
                          .rearrange("(j p) n -> p j n", p=128))
        ckF = consts.tile([128, KCAT, 2, 128], BF16, tag="ckF")
        ckgF = consts.tile([128, KCAT - 4, 2, 128], BF16, tag="ckgF")
        nc.sync.dma_start(out=ckF[:, 0:4, :, :], in_=P["xT_bf_full"].ap()[:, :]
                          .rearrange("(j p) (o n) -> p j o n", p=128, n=128))
        nc.sync.dma_start(out=ckF[:, 4:12, :, :], in_=P["cT_bf_full"].ap()[:, :]
                          .rearrange("(j p) (o n) -> p j o n", p=128, n=128))
        s_sb = consts.tile([128, 8, 128], F32, tag="s_sb")
        nc.sync.dma_start(out=s_sb[:], in_=P["s_sel"].ap()[:, :]
                          .rearrange("(j p) n -> p j n", p=128))
        ccol_sb = consts.tile([128, 2, 128], F32, tag="ccol_sb")
        nc.sync.dma_start(out=ccol_sb[:], in_=P["c_cols"].ap()[:, :]
                          .rearrange("(o p) n -> p o n", p=128))

        # ---------- selection: LN(s~) + gumbel, top-8 ----------
        nc.gpsimd.dma_start(out=s_bm[:],
                            in_=score_d.ap().rearrange("(b m) -> b m", b=BC))
        nc.vector.tensor_tensor(out=s_bm[:], in0=s_bm[:], in1=p100[:],
                                op=ALU.subtract)
        nc.gpsimd.dma_start(out=dbg_d.ap()[:], in_=s_bm[:])
        stats = bnpool.tile([BC, 1, 6], F32, tag="bn_stats", name="bn_stats")
        nc.vector.bn_stats(out=stats[:, 0, :], in_=s_bm[:])
        mv = bnpool.tile([BC, 2], F32, tag="bn_mv", name="mv")
        nc.vector.bn_aggr(out=mv[:], in_=stats[:])
        vpe = bnpool.tile([BC, 1], F32, tag="bn_vpe", name="vpe")
        nc.vector.tensor_scalar(out=vpe[:], in0=mv[:, 1:2], scalar1=1.0,
                                scalar2=EPS, op0=ALU.mult, op1=ALU.add)
        rstd = bnpool.tile([BC, 1], F32, tag="bn_rstd", name="rstd")
        rsqrt_newton(rstd[:], vpe[:], [BC, 1], "sel")
        tsel = consts.tile([BC, M], F32, tag="tsel")
        nc.vector.tensor_scalar(out=tsel[:], in0=s_bm[:], scalar1=mv[:, 0:1],
                                scalar2=rstd[:], op0=ALU.subtract, op1=ALU.mult)
        nc.vector.tensor_tensor(out=tsel[:], in0=tsel[:], in1=gt[:],
                                op=ALU.subtract)
        mx8 = consts.tile([BC, 8], F32, tag="mx8")
        nc.vector.max(out=mx8[:], in_=tsel[:])
        mi8 = consts.tile([BC, 8], U32, tag="mi8")
        nc.vector.max_index(out=mi8[:], in_max=mx8[:], in_values=tsel[:])

        # candidate row offsets -> [128, 1] via DRAM bounce
        off4 = consts.tile([BC, KSC], U32, tag="off4")
        nc.vector.tensor_tensor(out=off4[:], in0=mi8[:, 0:KSC],
                                in1=rowb[:, 0:1].to_broadcast([BC, KSC]),
                                op=ALU.add)
        nc.gpsimd.dma_start(out=off_bounce.ap().rearrange("(b k) -> b k", b=BC),
                            in_=off4[:])
        off128 = consts.tile([128, 1], U32, tag="off128")
        nc.gpsimd.dma_start(out=off128[:],
                            in_=off_bounce.ap().rearrange("(p a) -> p a", a=1))

        # gather candidate rows (hmem | keys | -100*prev), transpose
        Gc = scpool.tile([128, NAUG * 128], F32, tag="scratch", name="Gc")
        nc.gpsimd.indirect_dma_start(
            out=Gc[:], out_offset=None, in_=P["hmem_aug"].ap(),
            in_offset=bass.IndirectOffsetOnAxis(ap=off128[:, :1], axis=0))
        GT = scpool.tile([128, NAUG, 128], F32, tag="scratch", name="GT")
        for t in range(NAUG):
            tpg = tp_ps.tile([128, 128], F32, tag="tp", name="tpg")
            nc.tensor.transpose(tpg[:], Gc[:, t * 128:(t + 1) * 128], ident[:])
            nc.vector.tensor_copy(out=GT[:, t, :], in_=tpg[:])

        # exact re-score of the KSC candidates (fp32)
        spc = tp_ps.tile([1, 128], F32, tag="tp", name="spc")
        for f in range(2):
            fs = slice(f * 128, (f + 1) * 128)
            pe = pre_ps.tile([128, 128], F32, tag="pre", name="pe")
            for kh in range(9):
                nc.tensor.matmul(pe[:], lhsT=fchx[:, kh, fs], rhs=GT[:, kh, :],
                                 start=(kh == 0), stop=(kh == 8))
            hfc = consts.tile([128, 128], F32, tag=f"hfc{f}")
            nc.vector.tensor_tensor(
                out=hfc[:].rearrange("p (b k) -> p b k", b=BC),
                in0=pe[:].rearrange("p (b k) -> p b k", b=BC),
                in1=qT[:, f, :, None].to_broadcast([128, BC, KSC]),
                op=ALU.add)
            nc.scalar.activation(out=hfc[:], in_=hfc[:], func=AF.Tanh,
                                 bias=fcb[:, f:f + 1], scale=1.0)
            nc.tensor.matmul(spc[:], lhsT=veca[:, f:f + 1], rhs=hfc[:],
                             start=(f == 0), stop=(f == 1))
        scand = consts.tile([1, 128], F32, tag="scand")
        nc.vector.tensor_copy(out=scand[:], in_=spc[:])
        nc.vector.tensor_tensor(out=scand[:], in0=scand[:], in1=GT[0:1, 9, :],
                                op=ALU.add)
        nc.gpsimd.dma_start(out=sc_bounce.ap().rearrange("(a n) -> a n", a=1),
                            in_=scand[:])
        scand4 = consts.tile([BC, KSC], F32, tag="scand4")
        nc.gpsimd.dma_start(out=scand4[:],
                            in_=sc_bounce.ap().rearrange("(b k) -> b k", b=BC))

        # overlap slot: mm1 partials over x/c while the tail DMAs run
        ps1 = [ps1_ps.tile([128, 256], F32, tag="ps1", name=f"ps1_{bt}")
               for bt in range(2)]
        for bt in range(2):
            for k in range(12):
                nc.tensor.matmul(ps1[bt][:], lhsT=ckF[:, k, bt, :],
                                 rhs=w1sb[:, k, :], start=(k == 0), stop=False)

        # mix the exact candidate scores back in
        mi8f = consts.tile([BC, 8], F32, tag="mi8f")
        nc.vector.tensor_copy(out=mi8f[:], in_=mi8[:])
        for k in range(KSC):
            mask = bnpool.tile([BC, M], U32, tag="mask", name="mask")
            nc.vector.tensor_scalar(out=mask[:], in0=iota[:],
                                    scalar1=mi8f[:, k:k + 1], scalar2=None,
                                    op0=ALU.is_equal)
            nc.vector.copy_predicated(s_bm[:], mask[:],
                                      scand4[:, k:k + 1].to_broadcast([BC, M]))

        # final LN + gumbel + argmax + gather
        stats2 = bnpool.tile([BC, 1, 6], F32, tag="bn_stats", name="stats2")
        nc.vector.bn_stats(out=stats2[:, 0, :], in_=s_bm[:])
        mv2 = bnpool.tile([BC, 2], F32, tag="bn_mv", name="mv2")
        nc.vector.bn_aggr(out=mv2[:], in_=stats2[:])
        vpe2 = bnpool.tile([BC, 1], F32, tag="bn_vpe", name="vpe2")
        nc.vector.tensor_scalar(out=vpe2[:], in0=mv2[:, 1:2], scalar1=1.0,
                                scalar2=EPS, op0=ALU.mult, op1=ALU.add)
        rstd2 = bnpool.tile([BC, 1], F32, tag="bn_rstd", name="rstd2")
        rsqrt_newton(rstd2[:], vpe2[:], [BC, 1], "sel")
        ttot = consts.tile([BC, M], F32, tag="ttot")
        nc.vector.tensor_scalar(out=ttot[:], in0=s_bm[:], scalar1=mv2[:, 0:1],
                                scalar2=rstd2[:], op0=ALU.subtract, op1=ALU.mult)
        nc.vector.tensor_tensor(out=ttot[:], in0=ttot[:], in1=gt[:],
                                op=ALU.subtract)
        mxf = consts.tile([BC, 8], F32, tag="mxf")
        nc.vector.max(out=mxf[:], in_=ttot[:])
        mif = consts.tile([BC, 8], U32, tag="mif")
        nc.vector.max_index(out=mif[:], in_max=mxf[:], in_values=ttot[:])
        flat = consts.tile([BC, 1], U32, tag="flat")
        nc.vector.tensor_tensor(out=flat[:], in0=rowb[:], in1=mif[:, 0:1],
                                op=ALU.add)
        # ---------- AllGather the argmax indices, gather full batch locally ----
        nc.gpsimd.dma_start(out=agi_in.ap()[:], in_=flat[:])
        nc.gpsimd.collective_compute(
            "AllGather", ALU.bypass, replica_groups=GRP,
            ins=[agi_in.ap()[:]], outs=[agi_out.ap()[:]])
        flat_all = consts.tile([128, 2], U32, tag="flat_all")
        nc.gpsimd.dma_start(out=flat_all[:], in_=agi_out.ap()
                            .rearrange("(o p) a -> p (o a)", p=128))
        he_nat = scpool.tile([128, 2, NAUG * 128], F32, tag="scratch2",
                             name="he_nat", bufs=1)
        for bt in range(2):
            nc.gpsimd.indirect_dma_start(
                out=he_nat[:, bt, :], out_offset=None, in_=P["hmem_aug"].ap(),
                in_offset=bass.IndirectOffsetOnAxis(ap=flat_all[:, bt:bt + 1],
                                                    axis=0))
        heT = consts.tile([128, 8, 2, 128], F32, tag="heT")
        for bt in range(2):
            for kh in range(8):
                tph = tp_ps.tile([128, 128], F32, tag="tp", name="tph")
                nc.tensor.transpose(tph[:], he_nat[:, bt, kh * 128:(kh + 1) * 128],
                                    ident[:])
                nc.vector.tensor_copy(out=heT[:, kh, bt, :], in_=tph[:])
                nc.vector.tensor_copy(out=ckF[:, 12 + kh, bt, :], in_=tph[:])

        # r-path selection of own h_entry columns (independent of z path)
        sel_ps = tp_ps.tile([128, 256], F32, tag="tp", name="sel_ps")
        for bt in range(2):
            for kh in range(8):
                nc.tensor.matmul(sel_ps[:, bt * 128:(bt + 1) * 128],
                                 lhsT=s_sb[:, kh, :], rhs=heT[:, kh, bt, :],
                                 start=(kh == 0), stop=(kh == 7))
        selT = consts.tile([128, 256], F32, tag="selT")
        nc.vector.tensor_copy(out=selT[:], in_=sel_ps[:])
        hec = consts.tile([128, 2, 128], F32, tag="hec")
        for bt in range(2):
            tpc = tp_ps.tile([128, 128], F32, tag="tp", name="tpc")
            nc.tensor.transpose(tpc[:], selT[:, bt * 128:(bt + 1) * 128], ident[:])
            nc.vector.tensor_copy(out=hec[:, bt, :], in_=tpc[:])
        nc.scalar.activation(out=hec[:], in_=hec[:], func=AF.Tanh)

        # ---------- matmul 1: h_entry part, then LN3 via AllReduce ----------
        z1c = consts.tile([128, 2, 256], F32, tag="z1c")
        for bt in range(2):
            for k in range(12, KCAT):
                nc.tensor.matmul(ps1[bt][:], lhsT=ckF[:, k, bt, :],
                                 rhs=w1sb[:, k, :], start=False,
                                 stop=(k == KCAT - 1))
            nc.vector.tensor_copy(out=z1c[:, bt, :], in_=ps1[bt][:])
        z1sq = scpool.tile([128, 2, 256], F32, tag="scratch", name="z1sq")
        nc.vector.tensor_mul(out=z1sq[:], in0=z1c[:], in1=z1c[:])
        st3 = consts.tile([128, 2, 4], F32, tag="st3")
        nc.vector.reduce_sum(out=st3[:, :, 0:2],
                             in_=z1c[:].rearrange("p o (c n) -> p o c n", c=2),
                             axis=AX.X)
        nc.vector.reduce_sum(out=st3[:, :, 2:4],
                             in_=z1sq[:].rearrange("p o (c n) -> p o c n", c=2),
                             axis=AX.X)
        nc.gpsimd.dma_start(out=ar3_in.ap().rearrange("(o p) s -> p o s", p=128),
                            in_=st3[:])
        nc.gpsimd.collective_compute(
            "AllReduce", ALU.add, replica_groups=GRP,
            ins=[ar3_in.ap()[:]], outs=[ar3_out.ap()[:]])
        st3r = consts.tile([128, 2, 4], F32, tag="st3r")
        nc.gpsimd.dma_start(out=st3r[:], in_=ar3_out.ap()
                            .rearrange("(o p) s -> p o s", p=128))

        def ln_batch(z_view, s_ap, sq_ap, st_shape, bc_view_mean, tag):
            """z = (z - mean)*rstd for all chunks at once (vector only).
            s_ap/sq_ap: AllReduced sum/sumsq APs of shape st_shape.
            bc_view_mean: fn mapping a stats tile -> broadcast AP over z_view."""
            mean = bnpool.tile(st_shape, F32, tag=f"{tag}_m", name="lnm")
            nc.vector.tensor_scalar(out=mean[:], in0=s_ap, scalar1=1.0 / 1024,
                                    scalar2=None, op0=ALU.mult)
            vpe = bnpool.tile(st_shape, F32, tag=f"{tag}_v", name="lnv")
            nc.vector.tensor_scalar(out=vpe[:], in0=sq_ap, scalar1=1.0 / 1024,
                                    scalar2=EPS, op0=ALU.mult, op1=ALU.add)
            msq = bnpool.tile(st_shape, F32, tag=f"{tag}_q", name="lnq")
            nc.vector.tensor_tensor(out=msq[:], in0=mean[:], in1=mean[:],
                                    op=ALU.mult)
            nc.vector.tensor_tensor(out=vpe[:], in0=vpe[:], in1=msq[:],
                                    op=ALU.subtract)
            rstd = bnpool.tile(st_shape, F32, tag=f"{tag}_r", name="lnr")
            rsqrt_newton(rstd[:], vpe[:], st_shape, tag)
            nc.vector.tensor_tensor(out=z_view, in0=z_view,
                                    in1=bc_view_mean(mean), op=ALU.subtract)
            nc.vector.tensor_tensor(out=z_view, in0=z_view,
                                    in1=bc_view_mean(rstd), op=ALU.mult)

        g1c = consts.tile([128, 2, 256], F32, tag="g1c")
        nc.vector.tensor_copy(out=g1c[:], in_=z1c[:])
        ln_batch(g1c[:].rearrange("p o (c n) -> p o c n", c=2),
                 st3r[:, :, 0:2], st3r[:, :, 2:4], [128, 2, 2],
                 lambda t: t[:, :, :, None].to_broadcast([128, 2, 2, 128]),
                 "ln3")
        nc.scalar.activation(out=g1c[:], in_=g1c[:], func=AF.Sigmoid)

        # ---------- AllGather the transposed gate (bf16) ----------
        g1to = consts.tile([128, 2, 2, 128], BF16, tag="g1to")
        for lt in range(2):
            for bt in range(2):
                tpg2 = tp_ps.tile([128, 128], F32, tag="tp", name="tpg2")
                nc.tensor.transpose(tpg2[:], g1c[:, bt, lt * 128:(lt + 1) * 128],
                                    ident[:])
                nc.vector.tensor_copy(out=g1to[:, lt, bt, :], in_=tpg2[:])
        nc.gpsimd.dma_start(
            out=ag_g1_in.ap().rearrange("(lt p) (bt n) -> p lt bt n", p=128, n=128),
            in_=g1to[:])
        nc.gpsimd.collective_compute(
            "AllGather", ALU.bypass, replica_groups=GRP,
            ins=[ag_g1_in.ap()[:]], outs=[ag_g1_out.ap()[:]])
        g1T_sb = consts.tile([128, 16, 2, 128], BF16, tag="g1T_sb")
        nc.gpsimd.dma_start(out=g1T_sb[:], in_=ag_g1_out.ap()
                            .rearrange("(T p) (o n) -> p T o n", p=128, n=128))

        # gating: t<8 gates c, t>=8 gates h_entry
        for t in range(16):
            T = 2 * t if t < 8 else 2 * (t - 8) + 1
            for bt in range(2):
                src = ckF[:, 4 + t, bt, :] if t < 8 else heT[:, t - 8, bt, :]
                nc.vector.tensor_mul(out=ckgF[:, t, bt, :], in0=src,
                                     in1=g1T_sb[:, T, bt, :])

        # ---------- matmul 2: z = gated @ Wf cols, LN1 via AllReduce ----------
        z_sb = consts.tile([128, 2, 5, 128], F32, tag="z_sb")
        for bt in range(2):
            ps2 = ps2_ps.tile([128, 640], F32, tag="ps2", name=f"ps2_{bt}")
            for k in range(KCAT):
                lh = ckF[:, k, bt, :] if k < 4 else ckgF[:, k - 4, bt, :]
                nc.tensor.matmul(ps2[:, 0:512], lhsT=lh,
                                 rhs=wfsb[:, k, 0:512],
                                 start=(k == 0), stop=(k == KCAT - 1))
                nc.tensor.matmul(ps2[:, 512:640], lhsT=lh,
                                 rhs=wfsb[:, k, 512:640],
                                 start=(k == 0), stop=(k == KCAT - 1))
            nc.vector.tensor_copy(out=z_sb[:, bt, :, :]
                                  .rearrange("p c n -> p (c n)"), in_=ps2[:])
        zsq = scpool.tile([128, 2, 5, 128], F32, tag="scratch", name="zsq")
        nc.vector.tensor_mul(out=zsq[:], in0=z_sb[:], in1=z_sb[:])
        st1 = consts.tile([128, 2, 10], F32, tag="st1")
        nc.vector.reduce_sum(out=st1[:, :, 0:5], in_=z_sb[:], axis=AX.X)
        nc.vector.reduce_sum(out=st1[:, :, 5:10], in_=zsq[:], axis=AX.X)
        nc.gpsimd.dma_start(out=ar1_in.ap().rearrange("(o p) s -> p o s", p=128),
                            in_=st1[:])
        nc.gpsimd.collective_compute(
            "AllReduce", ALU.add, replica_groups=GRP,
            ins=[ar1_in.ap()[:]], outs=[ar1_out.ap()[:]])
        st1r = consts.tile([128, 2, 10], F32, tag="st1r")
        nc.gpsimd.dma_start(out=st1r[:], in_=ar1_out.ap()
                            .rearrange("(o p) s -> p o s", p=128))
        ln_batch(z_sb[:], st1r[:, :, 0:5], st1r[:, :, 5:10], [128, 2, 5],
                 lambda t: t[:, :, :, None].to_broadcast([128, 2, 5, 128]),
                 "ln1")

        # ---------- cell math on own columns ----------
        zi = z_sb[:, :, 0, :]; zj = z_sb[:, :, 1, :]; zf = z_sb[:, :, 2, :]
        zo = z_sb[:, :, 3, :]; zom = z_sb[:, :, 4, :]
        nc.scalar.activation(out=zf, in_=zf, func=AF.Sigmoid, bias=F_BIAS)
        nc.scalar.activation(out=zi, in_=zi, func=AF.Sigmoid)
        nc.scalar.activation(out=zo, in_=zo, func=AF.Sigmoid)
        nc.scalar.activation(out=zom, in_=zom, func=AF.Sigmoid)
        nc.scalar.activation(out=zj, in_=zj, func=AF.Tanh)
        # r-half of the output is independent of ln2 - ship it early
        out_sb = consts.tile([128, 2, 2, 128], F32, tag="out_sb")
        nc.vector.tensor_mul(out=out_sb[:, :, 1, :], in0=hec[:], in1=zom)
        nc.sync.dma_start(
            out=out_d.ap().rearrange("(o p) (h n) -> p o h n",
                                     p=128, n=128)[:, :, 1, :],
            in_=out_sb[:, :, 1, :])
        nc.vector.tensor_mul(out=zf, in0=ccol_sb[:], in1=zf)
        nc.vector.tensor_mul(out=zi, in0=zi, in1=zj)
        nc.vector.tensor_add(out=zf, in0=zf, in1=zi)
        ncsq = scpool.tile([128, 2, 128], F32, tag="scratch", name="ncsq")
        nc.vector.tensor_mul(out=ncsq[:], in0=zf, in1=zf)
        st2 = consts.tile([128, 2, 2], F32, tag="st2")
        nc.vector.reduce_sum(out=st2[:, :, 0:1], in_=zf, axis=AX.X)
        nc.vector.reduce_sum(out=st2[:, :, 1:2], in_=ncsq[:], axis=AX.X)
        nc.gpsimd.dma_start(out=ar2_in.ap().rearrange("(o p) s -> p o s", p=128),
                            in_=st2[:])
        nc.gpsimd.collective_compute(
            "AllReduce", ALU.add, replica_groups=GRP,
            ins=[ar2_in.ap()[:]], outs=[ar2_out.ap()[:]])
        st2r = consts.tile([128, 2, 2], F32, tag="st2r")
        nc.gpsimd.dma_start(out=st2r[:], in_=ar2_out.ap()
                            .rearrange("(o p) s -> p o s", p=128))
        ln_batch(zf, st2r[:, :, 0:1], st2r[:, :, 1:2], [128, 2, 1],
                 lambda t: t[:, :, 0:1].to_broadcast([128, 2, 128]), "ln2")
        nc.scalar.activation(out=zj, in_=zf, func=AF.Tanh)
        nc.vector.tensor_mul(out=out_sb[:, :, 0, :], in0=zj, in1=zo)
        nc.sync.dma_start(
            out=out_d.ap().rearrange("(o p) (h n) -> p o h n",
                                     p=128, n=128)[:, :, 0, :],
            in_=out_sb[:, :, 0, :])

    nc.compile()
    return nc


_NC = None


def _get_nc():
    global _NC
    if _NC is None:
        _NC = build_nc()
    return _NC


def make_in_maps(inputs):
    inp = {k: np.asarray(v) for k, v in inputs.items()}
    x = inp["x"].astype(np.float32)
    c = inp["c"].astype(np.float32)
    hmem = inp["hmem"].astype(np.float32)
    bf = ml_dtypes.bfloat16
    fc_w = inp["fc_w"].astype(np.float32)
    keys = inp["keys"].astype(np.float32)

    keysT_pad = np.zeros((128, M), np.float32)
    keysT_pad[:KD] = keys.T
    prev_full = inp["prev_read_location"].astype(np.float32)
    aug = np.zeros((B * M, NAUG * 128), np.float32)
    aug[:, 0:H] = hmem.reshape(B * M, H)
    aug[:, H:H + KD] = np.tile(keys, (B, 1))
    aug[:, H + 128] = -100.0 * prev_full.reshape(B * M)
    fchx = np.zeros((H + 128, F), np.float32)
    fchx[0:H] = fc_w[X + H + KD:X + H + KD + H]       # hmem rows
    fchx[H:H + KD] = fc_w[X + H:X + H + KD]           # keys rows (padded)

    W1 = inp["W_full1"].astype(np.float32)
    WF = inp["W_full"].astype(np.float32)

    shared = dict(
        keysT_pad=keysT_pad, hmem_aug=aug,
        fcxc=np.ascontiguousarray(fc_w[0:X + H]),
        fchx=fchx,
        fcu=np.ascontiguousarray(fc_w[X + 2 * H + KD:]),
        fcbp=inp["fc_b"].astype(np.float32),
        veca=inp["vec_a"].astype(np.float32).reshape(F),
        veca_r=inp["vec_a"].astype(np.float32).reshape(F),
        fchx_r=np.ascontiguousarray(fchx[0:H]),
        iota_m=np.arange(M, dtype=np.float32),
        xT_bf_full=np.ascontiguousarray(x.T).astype(bf),
        cT_bf_full=np.ascontiguousarray(c.T).astype(bf),
    )

    in_maps = []
    for cid in range(N_CORES):
        b0 = cid * BC
        xs = x[b0:b0 + BC]
        cs = c[b0:b0 + BC]
        hs = hmem[b0:b0 + BC]                              # [BC, M, H]
        prev = inp["prev_read_location"][b0:b0 + BC].astype(np.float32)
        m = dict(shared)
        m["hmemT"] = np.ascontiguousarray(hs.transpose(2, 0, 1).reshape(H, BM))
        m["row_base"] = ((cid * BC + np.arange(BC, dtype=np.uint32)) * M
                         ).reshape(BC, 1)
        m["xT"] = np.ascontiguousarray(xs.T)
        m["cT"] = np.ascontiguousarray(cs.T)
        m["u_t"] = inp["u_t"][b0:b0 + BC].astype(np.float32)
        m["prev"] = prev
        m["gumbel_u"] = inp["gumbel_u"][b0:b0 + BC].astype(np.float32)
        w1c = np.concatenate(
            [W1[:, ch * H + cid * 128:(ch * H + (cid + 1) * 128)]
             for ch in range(2)], axis=1)
        m["w1_cols"] = np.ascontiguousarray(w1c).astype(bf)
        wfc = np.concatenate(
            [WF[:, ch * H + cid * 128:(ch * H + (cid + 1) * 128)]
             for ch in range(5)], axis=1)
        m["wf_cols"] = np.ascontiguousarray(wfc).astype(bf)
        m["c_cols"] = np.ascontiguousarray(c[:, cid * 128:(cid + 1) * 128])
        sel = np.zeros((H, 128), np.float32)
        sel[cid * 128 + np.arange(128), np.arange(128)] = 1.0
        m["s_sel"] = sel
        in_maps.append(m)
    return in_maps


_LAST_EXEC_NS = None
_LAST_DBG = None


def kernel(**inputs):
    global _LAST_EXEC_NS, _LAST_DBG
    import os
    nc = _get_nc()
    in_maps = make_in_maps(inputs)
    trace = bool(int(os.environ.get("KERNEL_TRACE", "0")))
    res = run_bass_kernel_spmd(nc, in_maps, list(range(N_CORES)), trace=trace)
    _LAST_EXEC_NS = res.exec_time_ns
    _LAST_DBG = np.concatenate([res.results[i]["dbg_s"] for i in range(N_CORES)],
                               axis=0)
    outs = [res.results[i]["out"] for i in range(N_CORES)]
    full = np.zeros((B, 2 * H), np.float32)
    for cid in range(N_CORES):
        full[:, cid * 128:(cid + 1) * 128] = outs[cid][:, 0:128]
        full[:, H + cid * 128:H + (cid + 1) * 128] = outs[cid][:, 128:256]
    return full
